# revision 40
# baseline (speedup 1.0000x reference)
"""ApproxNDCGLoss Trainium2 kernel v2 (8 NeuronCores, data-parallel over graphs).

Math (per graph of G=128 candidates, labels binary):
  probs    = softmax(scores)            (no max-subtract: scores ~ N(0,1), fp32-safe)
  edcg     = sum_j probs_j * l_j * disc_j,   disc_j = 1/log2(j+2)
  idcg     = C(k), k = sum_j l_j, C = cumsum(disc)   (descending sort of binary
             labels == k ones first, so no sort needed)
  loss_g   = [k>0] * (1 - edcg/idcg);  loss = sum_g loss_g / B

Layout: the host hands each core its shard pre-TRANSPOSED so candidates sit on
the partition axis: [SUPERS=8, 128 cand, 2048 graphs] (full-width f32/i32 —
the device still streams all 16 MiB/core from HBM; only the layout changed).
This removes all PE transposes (the v1 kernel burned ~43us of PE sequencer on
512 data-as-weights Ldweights).

Per super-tile [128c x 2048g]:
  - gpsimd DMA loads scores f32->fp16 and labels i32->fp16 (cast in DMA)
  - ACT: eN = exp(s16)                   (plain softmax numerator)
  - DVE: eNL = eN * l16                  (fp16 2x mode)
  - PE, selector-weight accumulation: chunk q of 256 graphs, c = 8s+q,
    SEL[c%32] is [128,32] with column (c%32) = ones (or disc), so
       matmul(psX[rowbase:rowbase+32], SEL, rhs_chunk, start/stop)
    accumulates row c of a compact [64 rows x 256 graphs] PSUM tile:
       psD row c = denom_g = sum_c eN          (ones selector)
       psN row c = num_g   = sum_c eNL * disc  (disc-scaled selector)
       psK row c = k_g     = sum_c l           (ones selector)
    Weights never carry data -> only ~2 small Ldweights per chunk and the
    per-graph scalars land compact for phase 2.
  - phase 2 (once, [64 x 256]): 1/C(k) via degree-8 polynomial in ln k
    (max rel err 2.5e-5), ndcg = num*poly/denom, masked sum, one f32 matmul
    to reduce over partitions, scalar DMA out.
Host: shard + transpose inputs, sum the 8 partial scalars, / B.
`batch` is repeat(arange(B), G) by construction and is never read.
"""

import sys
from contextlib import ExitStack

import numpy as np

TRN_REPO = "/opt/trn_rl_repo"
if TRN_REPO not in sys.path:
    sys.path.insert(0, TRN_REPO)

import concourse.bass as bass
import concourse.mybir as mybir
import concourse.tile as tile

B = 131072
G = 128
NCORES = 8
BPC = B // NCORES            # graphs per core (16384)
SUPERS = 8                   # super-tiles per core
FREE = BPC // SUPERS         # graphs per super-tile (2048)
CH = 256                     # graphs per PE chunk
CPS = FREE // CH             # chunks per super (8)
NCHUNK = BPC // CH           # chunks per core (64) == compact rows
HALF = NCHUNK // 2           # accumulation-group boundary (32)
NWARM = 28                   # PE warm-up matmuls (pstate ramp)

F32 = mybir.dt.float32
F16 = mybir.dt.float16
I32 = mybir.dt.int32


def _fit_poly():
    """Degree-6 poly p(t) ~= 1/C(e^t), t = ln k (max rel err 4.5e-4 at
    k = 1..128; loss tolerance is 2e-2)."""
    disc = 1.0 / np.log2(np.arange(1, G + 1, dtype=np.float64) + 1.0)
    C = np.cumsum(disc)
    k = np.arange(1, G + 1, dtype=np.float64)
    t = np.log(k)
    g = 1.0 / C
    w = 1.0 / g
    deg = 5
    for _ in range(60):
        cf = np.polyfit(t, g, deg, w=w)
        rel = (np.polyval(cf, t) - g) / g
        w = w * (1 + 3 * np.abs(rel) / np.abs(rel).max())
    return [float(c) for c in cf]


POLY = _fit_poly()


def _make_consts():
    # disc_j for 0-based candidate j is 1/log2(j+2)
    disc = 1.0 / np.log2(np.arange(1, G + 1, dtype=np.float64) + 1.0)
    consts = np.zeros((128, 4), dtype=np.float32)
    consts[:, 0] = disc
    return consts


def _make_selo():
    """[128, 32*32] fp16: tile m*32.. holds the ones-selector for row (c%32):
    selo[p, 32*m + j] = 1.0 iff j == m."""
    selo = np.zeros((128, HALF * HALF), dtype=np.float16)
    for m in range(HALF):
        selo[:, HALF * m + m] = 1.0
    return selo


def _split_drain_waits(nc, max_waits=1):
    """Workaround: this neuronxcc build rejects instructions carrying more
    than ~1 sem wait ("Too many sync wait commands"). Hoist excess waits
    onto standalone InstEventSemaphore instructions issued immediately
    before, on the same engine queue (in-order, so semantics unchanged)."""
    ctr = 0
    for f in nc.m.functions:
        for blk in f.blocks:
            new_list = []
            for inst in blk.instructions:
                si = inst.sync_info
                if (
                    si is not None
                    and si.on_wait
                    and len(si.on_wait) > max_waits
                    and not isinstance(inst, mybir.InstEventSemaphore)
                ):
                    keep = si.on_wait[-max_waits:]
                    for wt in si.on_wait[:-max_waits]:
                        ctr += 1
                        ev = mybir.InstEventSemaphore(
                            name=f"hoistwait-{ctr}",
                            ins=[],
                            outs=[],
                            sync_info=mybir.SyncInfo(on_wait=[wt], on_update=[]),
                        )
                        ev.engine = inst.engine
                        new_list.append(ev)
                    si.on_wait = keep
                new_list.append(inst)
            blk.instructions = new_list


def build_nc(repeats=1):
    """repeats>1 unrolls the main pipeline R times over the same data
    (identical results) — used only for device-time measurement."""
    AF = mybir.ActivationFunctionType
    ALU = mybir.AluOpType
    AX = mybir.AxisListType

    nc = bass.Bass("TRN2", target_bir_lowering=False, debug=False, num_devices=NCORES)
    scores_d = nc.dram_tensor("scores", [SUPERS, 128, FREE], F32, kind="ExternalInput").ap()
    labels_d = nc.dram_tensor("labels", [SUPERS, 128, FREE], I32, kind="ExternalInput").ap()
    consts_d = nc.dram_tensor("consts", [128, 4], F32, kind="ExternalInput").ap()
    selo_d = nc.dram_tensor("selo", [128, HALF * HALF], F16, kind="ExternalInput").ap()
    out_d = nc.dram_tensor("out", [64, 2], F32, kind="ExternalOutput").ap()

    with tile.TileContext(nc) as tc:
        with ExitStack() as ctx:
            cpool = ctx.enter_context(tc.tile_pool(name="consts", bufs=1))
            cvec = cpool.tile([128, 4], F32)
            nc.sync.dma_start(cvec[:], consts_d[:])
            selo = cpool.tile([128, HALF * HALF], F16)
            seld = cpool.tile([128, HALF * HALF], F16)
            # PE pstate warm-up scratch
            wsrc = cpool.tile([128, HALF], F16)
            nc.vector.memset(wsrc[:], 0.0)
            rsrc = cpool.tile([128, CH], F16)
            nc.vector.memset(rsrc[:], 0.0)

            # compact per-graph scalars: group A rows [0:16] (chunks 0-15),
            # group B rows [32:48] (chunks 16-31) — PE output partition base
            # must be 0/32/64/96, so the two groups sit at bases 0 and 32
            pdp = ctx.enter_context(tc.tile_pool(name="cd", bufs=1, space="PSUM"))
            psD = pdp.tile([64, CH], F32)
            pnp = ctx.enter_context(tc.tile_pool(name="cn", bufs=1, space="PSUM"))
            psN = pnp.tile([64, CH], F32)
            pkp = ctx.enter_context(tc.tile_pool(name="ck", bufs=1, space="PSUM"))
            psK = pkp.tile([64, CH], F32)
            pwp = ctx.enter_context(tc.tile_pool(name="scr", bufs=1, space="PSUM"))
            pscr = pwp.tile([32, CH], F32)

            spool = ctx.enter_context(tc.tile_pool(name="s16", bufs=1))
            lpool = ctx.enter_context(tc.tile_pool(name="l16", bufs=1))
            epool = ctx.enter_context(tc.tile_pool(name="eN", bufs=1))
            e2pool = ctx.enter_context(tc.tile_pool(name="eNL", bufs=1))
            ph = ctx.enter_context(tc.tile_pool(name="ph", bufs=1))

            # phase-2 tiles, shared by the two row-group passes
            kc = ph.tile([64, CH], F32, tag="p2kc")
            tl = ph.tile([64, CH], F32, tag="p2tl")
            r = ph.tile([64, CH], F32, tag="p2r")
            w = ph.tile([64, CH], F32, tag="p2w")
            rd = ph.tile([64, CH], F32, tag="p2rd")
            qt = ph.tile([64, CH], F32, tag="p2qt")
            vt = ph.tile([64, CH], F32, tag="p2vt")
            colp = ph.tile([64, 1], F32, tag="p2colp")
            colv = ph.tile([64, 1], F32, tag="p2colv")

            # PE warm-up: ramp the tensor engine to full clock before the
            # real matmul stream arrives (cold-start runs at 0.65/1.2 GHz)
            for _wi in range(NWARM):
                nc.tensor.matmul(pscr[:], wsrc[:], rsrc[:], start=True, stop=True)

            HF = FREE // 2

            def kloop(s, l16):
                for q in range(CPS):
                    c = CPS * s + q
                    m = c % HALF
                    lo = 0 if c < HALF else 32
                    nc.tensor.matmul(
                        psK[lo : lo + 32, :], selo[:, m * 32 : (m + 1) * 32],
                        l16[:, q * CH : (q + 1) * CH],
                        start=(m == 0), stop=(m == HALF - 1), skip_group_check=True,
                    )

            def dloop(s, eN, qr=None):
                for q in qr if qr is not None else range(CPS):
                    c = CPS * s + q
                    m = c % HALF
                    lo = 0 if c < HALF else 32
                    nc.tensor.matmul(
                        psD[lo : lo + 32, :], selo[:, m * 32 : (m + 1) * 32],
                        eN[:, q * CH : (q + 1) * CH],
                        start=(m == 0), stop=(m == HALF - 1), skip_group_check=True,
                    )

            def nloop(s, eNL, qr=None):
                for q in qr if qr is not None else range(CPS):
                    c = CPS * s + q
                    m = c % HALF
                    lo = 0 if c < HALF else 32
                    nc.tensor.matmul(
                        psN[lo : lo + 32, :], seld[:, m * 32 : (m + 1) * 32],
                        eNL[:, q * CH : (q + 1) * CH],
                        start=(m == 0), stop=(m == HALF - 1), skip_group_check=True,
                    )

            def poly_chain(eng, rs):
                eng.tensor_scalar_mul(r[rs], tl[rs], float(POLY[0]))
                for cf in POLY[1:-1]:
                    eng.scalar_tensor_tensor(
                        r[rs], r[rs], float(cf), tl[rs], op0=ALU.add, op1=ALU.mult
                    )

            def prep_ops(eng, rs):
                """Off-critical pieces: rP = (r + P_last) * (1/den), and the
                valid-count column. Tail after the last num-matmul is then just
                qt = rP * psN -> reduce."""
                eng.scalar_tensor_tensor(
                    w[rs], r[rs], float(POLY[-1]), rd[rs], op0=ALU.add, op1=ALU.mult
                )
                nc.vector.reduce_sum(colv[rs], vt[rs], axis=AX.X)

            def tail_ops(eng, rs):
                eng.tensor_tensor(qt[rs], w[rs], psN[rs], op=ALU.mult)
                nc.vector.reduce_sum(colp[rs], qt[rs], axis=AX.X)

            for _rep in range(repeats):
                last = _rep == repeats - 1
                rsA, rsB = slice(0, 32), slice(32, 64)

                # ---- DMA stream (Pool SWDGE queue, this order) ----
                s16 = [
                    spool.tile([128, FREE], F16, name=f"s16_{i}", tag=f"s16_{i % 4}")
                    for i in range(SUPERS)
                ]
                l16 = [
                    lpool.tile([128, FREE], F16, name=f"l16_{i}", tag=f"l16_{i}")
                    for i in range(SUPERS)
                ]
                order = [("s", 0), ("l", 0), ("selo", 0), ("s", 1), ("l", 1),
                         ("s", 2), ("l", 2), ("s", 3), ("l", 3),
                         ("s", 4), ("l", 4), ("s", 5), ("l", 5), ("s", 6),
                         ("l", 6), ("l", 7), ("s7a", 0), ("s7b", 0)]
                for kind, i in order:
                    if kind == "s":
                        nc.gpsimd.dma_start(s16[i][:], scores_d[i])
                    elif kind == "l":
                        nc.gpsimd.dma_start(l16[i][:], labels_d[i])
                    elif kind == "selo":
                        if _rep == 0:
                            nc.gpsimd.dma_start(selo[:], selo_d[:])
                            nc.vector.tensor_scalar_mul(seld[:], selo[:], cvec[:, 0:1])
                    elif kind == "s7a":
                        nc.gpsimd.dma_start(s16[7][:, 0:HF], scores_d[7, :, 0:HF])
                    else:
                        nc.gpsimd.dma_start(s16[7][:, HF:FREE], scores_d[7, :, HF:FREE])

                # ---- main supers, half-tile compute granularity ----
                eN = {}
                eNL = {}
                for s in range(SUPERS):
                    eN[s] = epool.tile([128, FREE], F16, name=f"eN_{s}", tag=f"eN_{s % 3}")
                    eNL[s] = e2pool.tile([128, FREE], F16, name=f"eNL_{s}", tag=f"eNL_{s % 3}")
                    if s == 7 and last:
                        nc.vector.tensor_scalar_max(kc[rsB], psK[rsB], 1.0)
                        nc.vector.tensor_scalar(vt[rsB], psK[rsB], 0.5, None, op0=ALU.is_ge)
                    for h in range(2):
                        hsl = slice(h * HF, (h + 1) * HF)
                        nc.scalar.activation(
                            eN[s][:, hsl], s16[s][:, hsl], AF.Exp, bias=cvec[:, 1:2]
                        )
                        nc.vector.tensor_tensor(
                            eNL[s][:, hsl], eN[s][:, hsl], l16[s][:, hsl], op=ALU.mult
                        )
                        dloop(s, eN[s], qr=range(h * 4, h * 4 + 4))
                        nloop(s, eNL[s], qr=range(h * 4, h * 4 + 4))
                        if h == 0 and s < 7:
                            # K-matmuls sit between the half-tiles: labels land
                            # just after their scores (K7 rides with super 6)
                            kloop(s, l16[s])
                            if s == 6:
                                kloop(7, l16[7])
                        if s == 7 and h == 0 and last:
                            # psK-B closed at kloop(7); Ln fits between exp7a/b
                            nc.scalar.activation(
                                tl[rsB], kc[rsB], AF.Ln, bias=cvec[32:64, 1:2]
                            )
                            poly_chain(nc.vector, rsB)
                    if s == 3 and last:
                        # group A closed at the end of super 3
                        nc.vector.tensor_scalar_max(kc[rsA], psK[rsA], 1.0)
                        nc.scalar.activation(tl[rsA], kc[rsA], AF.Ln, bias=cvec[0:32, 1:2])
                        nc.vector.tensor_scalar(vt[rsA], psK[rsA], 0.5, None, op0=ALU.is_ge)
                        nc.vector.reciprocal(rd[rsA], psD[rsA])
                        poly_chain(nc.vector, rsA)
                    if s == 5 and last:
                        prep_ops(nc.vector, rsA)
                        tail_ops(nc.vector, rsA)  # group-A epilogue (PSUM: DVE only)

                if last:
                    nc.vector.reciprocal(rd[rsB], psD[rsB])
                    prep_ops(nc.vector, rsB)
                    tail_ops(nc.vector, rsB)  # tail-critical group-B epilogue

            outc = ph.tile([64, 2], F32, tag="p2outc")
            nc.vector.tensor_copy(outc[:, 0:1], colv[:])
            nc.vector.tensor_copy(outc[:, 1:2], colp[:])
            nc.sync.dma_start(out_d[:], outc[:])

    _split_drain_waits(nc)
    return nc


_NC_CACHE = None


def get_nc():
    global _NC_CACHE
    if _NC_CACHE is None:
        _NC_CACHE = build_nc()
    return _NC_CACHE


def make_in_maps(scores, labels):
    # per-core shard, then transpose so candidates sit on partitions:
    # [NCORES, SUPERS, 2048 graphs, 128 cand] -> [NCORES, SUPERS, 128, 2048]
    scores_sh = np.ascontiguousarray(
        np.asarray(scores, dtype=np.float32)
        .reshape(NCORES, SUPERS, FREE, G)
        .transpose(0, 1, 3, 2)
    )
    labels_sh = np.ascontiguousarray(
        np.asarray(labels, dtype=np.int32)
        .reshape(NCORES, SUPERS, FREE, G)
        .transpose(0, 1, 3, 2)
    )
    consts = _make_consts()
    selo = _make_selo()
    return [
        {"scores": scores_sh[c], "labels": labels_sh[c], "consts": consts, "selo": selo}
        for c in range(NCORES)
    ]


_RUNNER_CACHE = None


def _get_runner():
    """Compile the NEFF + jitted shard_map executor once per process."""
    global _RUNNER_CACHE
    if _RUNNER_CACHE is not None:
        return _RUNNER_CACHE

    import jax
    from jax.sharding import Mesh, PartitionSpec, NamedSharding
    from jax.experimental.shard_map import shard_map
    from concourse import bass2jax

    nc = get_nc()
    bass2jax.install_neuronx_cc_hook()
    partition_name = nc.partition_id_tensor.name if nc.partition_id_tensor else None
    in_names, out_names, out_avals, zero_outs = [], [], [], []
    for alloc in nc.m.functions[0].allocations:
        if not isinstance(alloc, mybir.MemoryLocationSet):
            continue
        name = alloc.memorylocations[0].name
        if alloc.kind == "ExternalInput":
            if name != partition_name:
                in_names.append(name)
        elif alloc.kind == "ExternalOutput":
            shape = tuple(alloc.tensor_shape)
            dtype = mybir.dt.np(alloc.dtype)
            out_names.append(name)
            out_avals.append(jax.core.ShapedArray(shape, dtype))
            zero_outs.append(np.zeros(shape, dtype))
    n_params = len(in_names)
    n_outs = len(out_avals)
    all_in_names = list(in_names) + list(out_names)
    if partition_name is not None:
        all_in_names.append(partition_name)

    def _body(*args):
        operands = list(args)
        if partition_name is not None:
            operands.append(bass2jax.partition_id_tensor())
        return tuple(
            bass2jax._bass_exec_p.bind(
                *operands,
                out_avals=tuple(out_avals),
                in_names=tuple(all_in_names),
                out_names=tuple(out_names),
                lowering_input_output_aliases=(),
                sim_require_finite=True,
                sim_require_nnan=True,
                nc=nc,
            )
        )

    devices = jax.devices()[:NCORES]
    mesh = Mesh(np.asarray(devices), ("core",))
    sharded = jax.jit(
        shard_map(
            _body,
            mesh=mesh,
            in_specs=(PartitionSpec("core"),) * (n_params + n_outs),
            out_specs=(PartitionSpec("core"),) * n_outs,
            check_rep=False,
        ),
        keep_unused=True,
    )
    sharding = NamedSharding(mesh, PartitionSpec("core"))

    def run(in_maps):
        concat_in = [
            np.concatenate(
                [np.asarray(in_maps[c][nm]) for c in range(NCORES)], axis=0
            )
            for nm in in_names
        ]
        concat_zeros = [
            np.zeros((NCORES * z.shape[0], *z.shape[1:]), z.dtype) for z in zero_outs
        ]
        dev_in = [jax.device_put(a, sharding) for a in concat_in]
        dev_zeros = [jax.device_put(a, sharding) for a in concat_zeros]
        outs = sharded(*dev_in, *dev_zeros)
        outs = [np.asarray(o) for o in outs]
        return {
            nm: outs[i].reshape(NCORES, *out_avals[i].shape) for i, nm in enumerate(out_names)
        }

    _RUNNER_CACHE = run
    return run


def reduce_out(out_concat):
    """[NCORES*64, 2] device output -> full loss sum: col0 = per-row valid
    counts, col1 = per-row sum of valid*ndcg; loss = sum(valid) - sum(ndcg)."""
    o = np.asarray(out_concat).reshape(NCORES, 64, 2)
    return float(o[..., 0].sum() - o[..., 1].sum())


def kernel(scores, labels, batch):
    run = _get_runner()
    in_maps = make_in_maps(scores, labels)
    outs = run(in_maps)
    total = reduce_out(outs["out"])
    return np.float32(total / B)


# revision 41
# speedup vs baseline: 1.0960x; 1.0960x over previous
"""ApproxNDCGLoss Trainium2 kernel v2 (8 NeuronCores, data-parallel over graphs).

Math (per graph of G=128 candidates, labels binary):
  probs    = softmax(scores)            (no max-subtract: scores ~ N(0,1), fp32-safe)
  edcg     = sum_j probs_j * l_j * disc_j,   disc_j = 1/log2(j+2)
  idcg     = C(k), k = sum_j l_j, C = cumsum(disc)   (descending sort of binary
             labels == k ones first, so no sort needed)
  loss_g   = [k>0] * (1 - edcg/idcg);  loss = sum_g loss_g / B

Layout: the host hands each core its shard pre-TRANSPOSED so candidates sit on
the partition axis: [SUPERS=8, 128 cand, 2048 graphs] (full-width f32/i32 —
the device still streams all 16 MiB/core from HBM; only the layout changed).
This removes all PE transposes (the v1 kernel burned ~43us of PE sequencer on
512 data-as-weights Ldweights).

Per super-tile [128c x 2048g]:
  - gpsimd DMA loads scores f32->fp16 and labels i32->fp16 (cast in DMA)
  - ACT: eN = exp(s16)                   (plain softmax numerator)
  - DVE: eNL = eN * l16                  (fp16 2x mode)
  - PE, selector-weight accumulation: chunk q of 256 graphs, c = 8s+q,
    SEL[c%32] is [128,32] with column (c%32) = ones (or disc), so
       matmul(psX[rowbase:rowbase+32], SEL, rhs_chunk, start/stop)
    accumulates row c of a compact [64 rows x 256 graphs] PSUM tile:
       psD row c = denom_g = sum_c eN          (ones selector)
       psN row c = num_g   = sum_c eNL * disc  (disc-scaled selector)
       psK row c = k_g     = sum_c l           (ones selector)
    Weights never carry data -> only ~2 small Ldweights per chunk and the
    per-graph scalars land compact for phase 2.
  - phase 2 (two row-groups, ops interleaved into idle engine windows):
    1/C(k) via degree-5 polynomial in ln k (max rel err 2.8e-3 vs 2e-2 tol),
    loss sum = sum(valid) - sum(num*poly(ln k)/denom) per row; the per-row
    [64,2] (valid-count, ndcg-sum) columns are DMA'd out and summed on host.
  - a short warm-up matmul burst ramps the PE clock (0.65->2.4 GHz pstate)
    before the real stream arrives.
Host: shard + transpose inputs, combine 8x[64,2] partials, / B.
`batch` is repeat(arange(B), G) by construction and is never read.
"""

import sys
from contextlib import ExitStack

import numpy as np

TRN_REPO = "/opt/trn_rl_repo"
if TRN_REPO not in sys.path:
    sys.path.insert(0, TRN_REPO)

import concourse.bass as bass
import concourse.mybir as mybir
import concourse.tile as tile

B = 131072
G = 128
NCORES = 8
BPC = B // NCORES            # graphs per core (16384)
SUPERS = 8                   # super-tiles per core
FREE = BPC // SUPERS         # graphs per super-tile (2048)
CH = 256                     # graphs per PE chunk
CPS = FREE // CH             # chunks per super (8)
NCHUNK = BPC // CH           # chunks per core (64) == compact rows
HALF = NCHUNK // 2           # accumulation-group boundary (32)
NWARM = 28                   # PE warm-up matmuls (pstate ramp)

F32 = mybir.dt.float32
F16 = mybir.dt.float16
I32 = mybir.dt.int32


def _fit_poly():
    """Degree-5 poly p(t) ~= 1/C(e^t), t = ln k (max rel err 2.8e-3 at
    k = 1..128; loss tolerance is 2e-2)."""
    disc = 1.0 / np.log2(np.arange(1, G + 1, dtype=np.float64) + 1.0)
    C = np.cumsum(disc)
    k = np.arange(1, G + 1, dtype=np.float64)
    t = np.log(k)
    g = 1.0 / C
    w = 1.0 / g
    deg = 5
    for _ in range(60):
        cf = np.polyfit(t, g, deg, w=w)
        rel = (np.polyval(cf, t) - g) / g
        w = w * (1 + 3 * np.abs(rel) / np.abs(rel).max())
    return [float(c) for c in cf]


POLY = _fit_poly()


def _make_consts():
    # disc_j for 0-based candidate j is 1/log2(j+2)
    disc = 1.0 / np.log2(np.arange(1, G + 1, dtype=np.float64) + 1.0)
    consts = np.zeros((128, 4), dtype=np.float32)
    consts[:, 0] = disc
    return consts


def _make_selo():
    """[128, 32*32] fp16: tile m*32.. holds the ones-selector for row (c%32):
    selo[p, 32*m + j] = 1.0 iff j == m."""
    selo = np.zeros((128, HALF * HALF), dtype=np.float16)
    for m in range(HALF):
        selo[:, HALF * m + m] = 1.0
    return selo


def _split_drain_waits(nc, max_waits=1):
    """Workaround: this neuronxcc build rejects instructions carrying more
    than ~1 sem wait ("Too many sync wait commands"). Hoist excess waits
    onto standalone InstEventSemaphore instructions issued immediately
    before, on the same engine queue (in-order, so semantics unchanged)."""
    ctr = 0
    for f in nc.m.functions:
        for blk in f.blocks:
            new_list = []
            for inst in blk.instructions:
                si = inst.sync_info
                if (
                    si is not None
                    and si.on_wait
                    and len(si.on_wait) > max_waits
                    and not isinstance(inst, mybir.InstEventSemaphore)
                ):
                    keep = si.on_wait[-max_waits:]
                    for wt in si.on_wait[:-max_waits]:
                        ctr += 1
                        ev = mybir.InstEventSemaphore(
                            name=f"hoistwait-{ctr}",
                            ins=[],
                            outs=[],
                            sync_info=mybir.SyncInfo(on_wait=[wt], on_update=[]),
                        )
                        ev.engine = inst.engine
                        new_list.append(ev)
                    si.on_wait = keep
                new_list.append(inst)
            blk.instructions = new_list


def build_nc(repeats=1):
    """repeats>1 unrolls the main pipeline R times over the same data
    (identical results) — used only for device-time measurement."""
    AF = mybir.ActivationFunctionType
    ALU = mybir.AluOpType
    AX = mybir.AxisListType

    nc = bass.Bass("TRN2", target_bir_lowering=False, debug=False, num_devices=NCORES)
    scores_d = nc.dram_tensor("scores", [SUPERS, 128, FREE], F32, kind="ExternalInput").ap()
    labels_d = nc.dram_tensor("labels", [SUPERS, 128, FREE], I32, kind="ExternalInput").ap()
    consts_d = nc.dram_tensor("consts", [128, 4], F32, kind="ExternalInput").ap()
    selo_d = nc.dram_tensor("selo", [128, HALF * HALF], F16, kind="ExternalInput").ap()
    out_d = nc.dram_tensor("out", [64, 2], F32, kind="ExternalOutput").ap()

    with tile.TileContext(nc) as tc:
        with ExitStack() as ctx:
            cpool = ctx.enter_context(tc.tile_pool(name="consts", bufs=1))
            cvec = cpool.tile([128, 4], F32)
            nc.sync.dma_start(cvec[:], consts_d[:])
            selo = cpool.tile([128, HALF * HALF], F16)
            seld = cpool.tile([128, HALF * HALF], F16)
            # PE pstate warm-up scratch
            wsrc = cpool.tile([128, HALF], F16)
            nc.vector.memset(wsrc[:], 0.0)
            rsrc = cpool.tile([128, CH], F16)
            nc.vector.memset(rsrc[:], 0.0)

            # compact per-graph scalars: group A rows [0:32] (chunks 0-31),
            # group B rows [32:64] (chunks 32-63) — PE output partition base
            # must be 0/32/64/96, so the two groups sit at bases 0 and 32
            pdp = ctx.enter_context(tc.tile_pool(name="cd", bufs=1, space="PSUM"))
            psD = pdp.tile([64, CH], F32)
            pnp = ctx.enter_context(tc.tile_pool(name="cn", bufs=1, space="PSUM"))
            psN = pnp.tile([64, CH], F32)
            pkp = ctx.enter_context(tc.tile_pool(name="ck", bufs=1, space="PSUM"))
            psK = pkp.tile([64, CH], F32)
            pwp = ctx.enter_context(tc.tile_pool(name="scr", bufs=1, space="PSUM"))
            pscr = pwp.tile([32, CH], F32)

            spool = ctx.enter_context(tc.tile_pool(name="s16", bufs=1))
            lpool = ctx.enter_context(tc.tile_pool(name="l16", bufs=1))
            epool = ctx.enter_context(tc.tile_pool(name="eN", bufs=1))
            e2pool = ctx.enter_context(tc.tile_pool(name="eNL", bufs=1))
            ph = ctx.enter_context(tc.tile_pool(name="ph", bufs=1))

            # phase-2 tiles, shared by the two row-group passes
            kc = ph.tile([64, CH], F32, tag="p2kc")
            tl = ph.tile([64, CH], F32, tag="p2tl")
            r = ph.tile([64, CH], F32, tag="p2r")
            w = ph.tile([64, CH], F32, tag="p2w")
            rd = ph.tile([64, CH], F32, tag="p2rd")
            qt = ph.tile([64, CH], F32, tag="p2qt")
            vt = ph.tile([64, CH], F32, tag="p2vt")
            colp = ph.tile([64, 1], F32, tag="p2colp")
            colv = ph.tile([64, 1], F32, tag="p2colv")

            # PE warm-up: ramp the tensor engine to full clock before the
            # real matmul stream arrives (cold-start runs at 0.65/1.2 GHz)
            for _wi in range(NWARM):
                nc.tensor.matmul(pscr[:], wsrc[:], rsrc[:], start=True, stop=True)

            HF = FREE // 2

            def kloop(s, l16):
                for q in range(CPS):
                    c = CPS * s + q
                    m = c % HALF
                    lo = 0 if c < HALF else 32
                    nc.tensor.matmul(
                        psK[lo : lo + 32, :], selo[:, m * 32 : (m + 1) * 32],
                        l16[:, q * CH : (q + 1) * CH],
                        start=(m == 0), stop=(m == HALF - 1), skip_group_check=True,
                    )

            def dloop(s, eN, qr=None):
                for q in qr if qr is not None else range(CPS):
                    c = CPS * s + q
                    m = c % HALF
                    lo = 0 if c < HALF else 32
                    nc.tensor.matmul(
                        psD[lo : lo + 32, :], selo[:, m * 32 : (m + 1) * 32],
                        eN[:, q * CH : (q + 1) * CH],
                        start=(m == 0), stop=(m == HALF - 1), skip_group_check=True,
                    )

            def nloop(s, eNL, qr=None):
                for q in qr if qr is not None else range(CPS):
                    c = CPS * s + q
                    m = c % HALF
                    lo = 0 if c < HALF else 32
                    nc.tensor.matmul(
                        psN[lo : lo + 32, :], seld[:, m * 32 : (m + 1) * 32],
                        eNL[:, q * CH : (q + 1) * CH],
                        start=(m == 0), stop=(m == HALF - 1), skip_group_check=True,
                    )

            def poly_chain(eng, rs):
                eng.tensor_scalar_mul(r[rs], tl[rs], float(POLY[0]))
                for cf in POLY[1:-1]:
                    eng.scalar_tensor_tensor(
                        r[rs], r[rs], float(cf), tl[rs], op0=ALU.add, op1=ALU.mult
                    )

            def prep_ops(eng, rs):
                """Off-critical pieces: rP = (r + P_last) * (1/den), and the
                valid-count column. Tail after the last num-matmul is then just
                qt = rP * psN -> reduce."""
                eng.scalar_tensor_tensor(
                    w[rs], r[rs], float(POLY[-1]), rd[rs], op0=ALU.add, op1=ALU.mult
                )
                nc.vector.reduce_sum(colv[rs], vt[rs], axis=AX.X)

            def tail_ops(eng, rs):
                eng.tensor_tensor(qt[rs], w[rs], psN[rs], op=ALU.mult)
                nc.vector.reduce_sum(colp[rs], qt[rs], axis=AX.X)

            for _rep in range(repeats):
                last = _rep == repeats - 1
                rsA, rsB = slice(0, 32), slice(32, 64)

                # ---- DMA stream (Pool SWDGE queue, this order) ----
                s16 = [
                    spool.tile([128, FREE], F16, name=f"s16_{i}", tag=f"s16_{i % 4}")
                    for i in range(SUPERS)
                ]
                l16 = [
                    lpool.tile([128, FREE], F16, name=f"l16_{i}", tag=f"l16_{i}")
                    for i in range(SUPERS)
                ]
                order = [("s", 0), ("l", 0), ("selo", 0), ("s", 1), ("l", 1),
                         ("s", 2), ("l", 2), ("s", 3), ("l", 3),
                         ("s", 4), ("l", 4), ("s", 5), ("l", 5), ("s", 6),
                         ("l", 6), ("l", 7), ("s7a", 0), ("s7b", 0)]
                for kind, i in order:
                    if kind == "s":
                        nc.gpsimd.dma_start(s16[i][:], scores_d[i])
                    elif kind == "l":
                        nc.gpsimd.dma_start(l16[i][:], labels_d[i])
                    elif kind == "selo":
                        if _rep == 0:
                            nc.gpsimd.dma_start(selo[:], selo_d[:])
                            nc.vector.tensor_scalar_mul(seld[:], selo[:], cvec[:, 0:1])
                    elif kind == "s7a":
                        nc.gpsimd.dma_start(s16[7][:, 0:HF], scores_d[7, :, 0:HF])
                    else:
                        nc.gpsimd.dma_start(s16[7][:, HF:FREE], scores_d[7, :, HF:FREE])

                # ---- main supers, half-tile compute granularity ----
                eN = {}
                eNL = {}
                for s in range(SUPERS):
                    eN[s] = epool.tile([128, FREE], F16, name=f"eN_{s}", tag=f"eN_{s % 3}")
                    eNL[s] = e2pool.tile([128, FREE], F16, name=f"eNL_{s}", tag=f"eNL_{s % 3}")
                    if s == 7 and last:
                        nc.vector.tensor_scalar_max(kc[rsB], psK[rsB], 1.0)
                        nc.vector.tensor_scalar(vt[rsB], psK[rsB], 0.5, None, op0=ALU.is_ge)
                    for h in range(2):
                        hsl = slice(h * HF, (h + 1) * HF)
                        nc.scalar.activation(
                            eN[s][:, hsl], s16[s][:, hsl], AF.Exp, bias=cvec[:, 1:2]
                        )
                        nc.vector.tensor_tensor(
                            eNL[s][:, hsl], eN[s][:, hsl], l16[s][:, hsl], op=ALU.mult
                        )
                        dloop(s, eN[s], qr=range(h * 4, h * 4 + 4))
                        nloop(s, eNL[s], qr=range(h * 4, h * 4 + 4))
                        if h == 0 and s < 7:
                            # K-matmuls sit between the half-tiles: labels land
                            # just after their scores (K7 rides with super 6)
                            kloop(s, l16[s])
                            if s == 6:
                                kloop(7, l16[7])
                        if s == 7 and h == 0 and last:
                            # psK-B closed at kloop(7); Ln fits between exp7a/b
                            nc.scalar.activation(
                                tl[rsB], kc[rsB], AF.Ln, bias=cvec[32:64, 1:2]
                            )
                            poly_chain(nc.vector, rsB)
                    if s == 3 and last:
                        # group A closed at the end of super 3
                        nc.vector.tensor_scalar_max(kc[rsA], psK[rsA], 1.0)
                        nc.scalar.activation(tl[rsA], kc[rsA], AF.Ln, bias=cvec[0:32, 1:2])
                        nc.vector.tensor_scalar(vt[rsA], psK[rsA], 0.5, None, op0=ALU.is_ge)
                        nc.vector.reciprocal(rd[rsA], psD[rsA])
                        poly_chain(nc.vector, rsA)
                    if s == 5 and last:
                        prep_ops(nc.vector, rsA)
                        tail_ops(nc.vector, rsA)  # group-A epilogue (PSUM: DVE only)

                if last:
                    nc.vector.reciprocal(rd[rsB], psD[rsB])
                    prep_ops(nc.vector, rsB)
                    tail_ops(nc.vector, rsB)  # tail-critical group-B epilogue

            outc = ph.tile([64, 2], F32, tag="p2outc")
            nc.vector.tensor_copy(outc[:, 0:1], colv[:])
            nc.vector.tensor_copy(outc[:, 1:2], colp[:])
            nc.sync.dma_start(out_d[:], outc[:])

    _split_drain_waits(nc)
    return nc


_NC_CACHE = None


def get_nc():
    global _NC_CACHE
    if _NC_CACHE is None:
        _NC_CACHE = build_nc()
    return _NC_CACHE


def make_in_maps(scores, labels):
    # per-core shard, then transpose so candidates sit on partitions:
    # [NCORES, SUPERS, 2048 graphs, 128 cand] -> [NCORES, SUPERS, 128, 2048]
    scores_sh = np.ascontiguousarray(
        np.asarray(scores, dtype=np.float32)
        .reshape(NCORES, SUPERS, FREE, G)
        .transpose(0, 1, 3, 2)
    )
    labels_sh = np.ascontiguousarray(
        np.asarray(labels, dtype=np.int32)
        .reshape(NCORES, SUPERS, FREE, G)
        .transpose(0, 1, 3, 2)
    )
    consts = _make_consts()
    selo = _make_selo()
    return [
        {"scores": scores_sh[c], "labels": labels_sh[c], "consts": consts, "selo": selo}
        for c in range(NCORES)
    ]


_RUNNER_CACHE = None


def _get_runner():
    """Compile the NEFF + jitted shard_map executor once per process."""
    global _RUNNER_CACHE
    if _RUNNER_CACHE is not None:
        return _RUNNER_CACHE

    import jax
    from jax.sharding import Mesh, PartitionSpec, NamedSharding
    from jax.experimental.shard_map import shard_map
    from concourse import bass2jax

    nc = get_nc()
    bass2jax.install_neuronx_cc_hook()
    partition_name = nc.partition_id_tensor.name if nc.partition_id_tensor else None
    in_names, out_names, out_avals, zero_outs = [], [], [], []
    for alloc in nc.m.functions[0].allocations:
        if not isinstance(alloc, mybir.MemoryLocationSet):
            continue
        name = alloc.memorylocations[0].name
        if alloc.kind == "ExternalInput":
            if name != partition_name:
                in_names.append(name)
        elif alloc.kind == "ExternalOutput":
            shape = tuple(alloc.tensor_shape)
            dtype = mybir.dt.np(alloc.dtype)
            out_names.append(name)
            out_avals.append(jax.core.ShapedArray(shape, dtype))
            zero_outs.append(np.zeros(shape, dtype))
    n_params = len(in_names)
    n_outs = len(out_avals)
    all_in_names = list(in_names) + list(out_names)
    if partition_name is not None:
        all_in_names.append(partition_name)

    def _body(*args):
        operands = list(args)
        if partition_name is not None:
            operands.append(bass2jax.partition_id_tensor())
        return tuple(
            bass2jax._bass_exec_p.bind(
                *operands,
                out_avals=tuple(out_avals),
                in_names=tuple(all_in_names),
                out_names=tuple(out_names),
                lowering_input_output_aliases=(),
                sim_require_finite=True,
                sim_require_nnan=True,
                nc=nc,
            )
        )

    devices = jax.devices()[:NCORES]
    mesh = Mesh(np.asarray(devices), ("core",))
    sharded = jax.jit(
        shard_map(
            _body,
            mesh=mesh,
            in_specs=(PartitionSpec("core"),) * (n_params + n_outs),
            out_specs=(PartitionSpec("core"),) * n_outs,
            check_rep=False,
        ),
        keep_unused=True,
    )
    sharding = NamedSharding(mesh, PartitionSpec("core"))

    def run(in_maps):
        concat_in = [
            np.concatenate(
                [np.asarray(in_maps[c][nm]) for c in range(NCORES)], axis=0
            )
            for nm in in_names
        ]
        concat_zeros = [
            np.zeros((NCORES * z.shape[0], *z.shape[1:]), z.dtype) for z in zero_outs
        ]
        dev_in = [jax.device_put(a, sharding) for a in concat_in]
        dev_zeros = [jax.device_put(a, sharding) for a in concat_zeros]
        outs = sharded(*dev_in, *dev_zeros)
        outs = [np.asarray(o) for o in outs]
        return {
            nm: outs[i].reshape(NCORES, *out_avals[i].shape) for i, nm in enumerate(out_names)
        }

    _RUNNER_CACHE = run
    return run


def reduce_out(out_concat):
    """[NCORES*64, 2] device output -> full loss sum: col0 = per-row valid
    counts, col1 = per-row sum of valid*ndcg; loss = sum(valid) - sum(ndcg)."""
    o = np.asarray(out_concat).reshape(NCORES, 64, 2)
    return float(o[..., 0].sum() - o[..., 1].sum())


def kernel(scores, labels, batch):
    run = _get_runner()
    in_maps = make_in_maps(scores, labels)
    outs = run(in_maps)
    total = reduce_out(outs["out"])
    return np.float32(total / B)


# revision 43
# speedup vs baseline: 1.2499x; 1.1405x over previous
"""ApproxNDCGLoss Trainium2 kernel v2 (8 NeuronCores, data-parallel over graphs).

Math (per graph of G=128 candidates, labels binary):
  probs    = softmax(scores)            (no max-subtract: scores ~ N(0,1), fp32-safe)
  edcg     = sum_j probs_j * l_j * disc_j,   disc_j = 1/log2(j+2)
  idcg     = C(k), k = sum_j l_j, C = cumsum(disc)   (descending sort of binary
             labels == k ones first, so no sort needed)
  loss_g   = [k>0] * (1 - edcg/idcg);  loss = sum_g loss_g / B

Layout: the host hands each core its shard pre-TRANSPOSED so candidates sit on
the partition axis: [SUPERS=8, 128 cand, 2048 graphs] (full-width f32/i32 —
the device still streams all 16 MiB/core from HBM; only the layout changed).
This removes all PE transposes (the v1 kernel burned ~43us of PE sequencer on
512 data-as-weights Ldweights).

Per super-tile [128c x 2048g]:
  - gpsimd DMA loads scores f32->fp16 and labels i32->fp16 (cast in DMA)
  - ACT: eN = exp(s16)                   (plain softmax numerator)
  - DVE: eNL = eN * l16                  (fp16 2x mode)
  - PE, selector-weight accumulation: chunk q of 256 graphs, c = 8s+q,
    SEL[c%32] is [128,32] with column (c%32) = ones (or disc), so
       matmul(psX[rowbase:rowbase+32], SEL, rhs_chunk, start/stop)
    accumulates row c of a compact [64 rows x 256 graphs] PSUM tile:
       psD row c = denom_g = sum_c eN          (ones selector)
       psN row c = num_g   = sum_c eNL * disc  (disc-scaled selector)
       psK row c = k_g     = sum_c l           (ones selector)
    Weights never carry data -> only ~2 small Ldweights per chunk and the
    per-graph scalars land compact for phase 2.
  - phase 2 (two row-groups, ops interleaved into idle engine windows):
    1/C(k) via degree-5 polynomial in ln k (max rel err 2.8e-3 vs 2e-2 tol),
    loss sum = sum(valid) - sum(num*poly(ln k)/denom) per row; the per-row
    [64,2] (valid-count, ndcg-sum) columns are DMA'd out and summed on host.
  - a short warm-up matmul burst ramps the PE clock (0.65->2.4 GHz pstate)
    before the real stream arrives.
Host: shard + transpose inputs, combine 8x[64,2] partials, / B.
`batch` is repeat(arange(B), G) by construction and is never read.
"""

import sys
from contextlib import ExitStack

import numpy as np

TRN_REPO = "/opt/trn_rl_repo"
if TRN_REPO not in sys.path:
    sys.path.insert(0, TRN_REPO)

import concourse.bass as bass
import concourse.mybir as mybir
import concourse.tile as tile

B = 131072
G = 128
NCORES = 8
BPC = B // NCORES            # graphs per core (16384)
SUPERS = 8                   # super-tiles per core
FREE = BPC // SUPERS         # graphs per super-tile (2048)
CH = 256                     # graphs per PE chunk
CPS = FREE // CH             # chunks per super (8)
NCHUNK = BPC // CH           # chunks per core (64) == compact rows
HALF = NCHUNK // 2           # accumulation-group boundary (32)
NWARM = 28                   # PE warm-up matmuls (pstate ramp)

F32 = mybir.dt.float32
F16 = mybir.dt.float16
I32 = mybir.dt.int32


def _fit_poly():
    """Degree-5 poly p(t) ~= 1/C(e^t), t = ln k (max rel err 2.8e-3 at
    k = 1..128; loss tolerance is 2e-2)."""
    disc = 1.0 / np.log2(np.arange(1, G + 1, dtype=np.float64) + 1.0)
    C = np.cumsum(disc)
    k = np.arange(1, G + 1, dtype=np.float64)
    t = np.log(k)
    g = 1.0 / C
    w = 1.0 / g
    deg = 5
    for _ in range(60):
        cf = np.polyfit(t, g, deg, w=w)
        rel = (np.polyval(cf, t) - g) / g
        w = w * (1 + 3 * np.abs(rel) / np.abs(rel).max())
    return [float(c) for c in cf]


POLY = _fit_poly()


def _make_consts():
    # disc_j for 0-based candidate j is 1/log2(j+2)
    disc = 1.0 / np.log2(np.arange(1, G + 1, dtype=np.float64) + 1.0)
    consts = np.zeros((128, 4), dtype=np.float32)
    consts[:, 0] = disc
    return consts


def _make_selo():
    """[128, 32*32] fp16: tile m*32.. holds the ones-selector for row (c%32):
    selo[p, 32*m + j] = 1.0 iff j == m."""
    selo = np.zeros((128, HALF * HALF), dtype=np.float16)
    for m in range(HALF):
        selo[:, HALF * m + m] = 1.0
    return selo


def _split_drain_waits(nc, max_waits=1):
    """Workaround: this neuronxcc build rejects instructions carrying more
    than ~1 sem wait ("Too many sync wait commands"). Hoist excess waits
    onto standalone InstEventSemaphore instructions issued immediately
    before, on the same engine queue (in-order, so semantics unchanged)."""
    ctr = 0
    for f in nc.m.functions:
        for blk in f.blocks:
            new_list = []
            for inst in blk.instructions:
                si = inst.sync_info
                if (
                    si is not None
                    and si.on_wait
                    and len(si.on_wait) > max_waits
                    and not isinstance(inst, mybir.InstEventSemaphore)
                ):
                    keep = si.on_wait[-max_waits:]
                    for wt in si.on_wait[:-max_waits]:
                        ctr += 1
                        ev = mybir.InstEventSemaphore(
                            name=f"hoistwait-{ctr}",
                            ins=[],
                            outs=[],
                            sync_info=mybir.SyncInfo(on_wait=[wt], on_update=[]),
                        )
                        ev.engine = inst.engine
                        new_list.append(ev)
                    si.on_wait = keep
                new_list.append(inst)
            blk.instructions = new_list


def build_nc(repeats=1):
    """repeats>1 unrolls the main pipeline R times over the same data
    (identical results) — used only for device-time measurement."""
    AF = mybir.ActivationFunctionType
    ALU = mybir.AluOpType
    AX = mybir.AxisListType

    nc = bass.Bass("TRN2", target_bir_lowering=False, debug=False, num_devices=NCORES)
    scores_d = nc.dram_tensor("scores", [SUPERS, 128, FREE], F32, kind="ExternalInput").ap()
    labels_d = nc.dram_tensor("labels", [SUPERS, 128, FREE], I32, kind="ExternalInput").ap()
    consts_d = nc.dram_tensor("consts", [128, 4], F32, kind="ExternalInput").ap()
    selo_d = nc.dram_tensor("selo", [128, HALF * HALF], F16, kind="ExternalInput").ap()
    out_d = nc.dram_tensor("out", [64, 2], F32, kind="ExternalOutput").ap()

    with tile.TileContext(nc) as tc:
        with ExitStack() as ctx:
            cpool = ctx.enter_context(tc.tile_pool(name="consts", bufs=1))
            cvec = cpool.tile([128, 4], F32)
            nc.sync.dma_start(cvec[:], consts_d[:])
            selo = cpool.tile([128, HALF * HALF], F16)
            seld = cpool.tile([128, HALF * HALF], F16)
            # PE pstate warm-up scratch
            wsrc = cpool.tile([128, HALF], F16)
            nc.vector.memset(wsrc[:], 0.0)
            rsrc = cpool.tile([128, CH], F16)
            nc.vector.memset(rsrc[:], 0.0)

            # compact per-graph scalars: group A rows [0:32] (chunks 0-31),
            # group B rows [32:64] (chunks 32-63) — PE output partition base
            # must be 0/32/64/96, so the two groups sit at bases 0 and 32
            pdp = ctx.enter_context(tc.tile_pool(name="cd", bufs=1, space="PSUM"))
            psD = pdp.tile([64, CH], F32)
            pnp = ctx.enter_context(tc.tile_pool(name="cn", bufs=1, space="PSUM"))
            psN = pnp.tile([64, CH], F32)
            pkp = ctx.enter_context(tc.tile_pool(name="ck", bufs=1, space="PSUM"))
            psK = pkp.tile([64, CH], F32)
            pwp = ctx.enter_context(tc.tile_pool(name="scr", bufs=1, space="PSUM"))
            pscr = pwp.tile([32, CH], F32)

            spool = ctx.enter_context(tc.tile_pool(name="s16", bufs=1))
            lpool = ctx.enter_context(tc.tile_pool(name="l16", bufs=1))
            epool = ctx.enter_context(tc.tile_pool(name="eN", bufs=1))
            e2pool = ctx.enter_context(tc.tile_pool(name="eNL", bufs=1))
            ph = ctx.enter_context(tc.tile_pool(name="ph", bufs=1))

            # phase-2 tiles, shared by the two row-group passes
            kc = ph.tile([64, CH], F32, tag="p2kc")
            tl = ph.tile([64, CH], F32, tag="p2tl")
            r = ph.tile([64, CH], F32, tag="p2r")
            w = ph.tile([64, CH], F32, tag="p2w")
            rd = ph.tile([64, CH], F32, tag="p2rd")
            qt = ph.tile([64, CH], F32, tag="p2qt")
            vt = ph.tile([64, CH], F32, tag="p2vt")
            colp = ph.tile([64, 1], F32, tag="p2colp")
            colv = ph.tile([64, 1], F32, tag="p2colv")

            # PE warm-up: ramp the tensor engine to full clock before the
            # real matmul stream arrives (cold-start runs at 0.65/1.2 GHz)
            for _wi in range(NWARM):
                nc.tensor.matmul(pscr[:], wsrc[:], rsrc[:], start=True, stop=True)

            HF = FREE // 2

            def kloop(s, l16):
                for q in range(CPS):
                    c = CPS * s + q
                    m = c % HALF
                    lo = 0 if c < HALF else 32
                    nc.tensor.matmul(
                        psK[lo : lo + 32, :], selo[:, m * 32 : (m + 1) * 32],
                        l16[:, q * CH : (q + 1) * CH],
                        start=(m == 0), stop=(m == HALF - 1), skip_group_check=True,
                    )

            def dloop(s, eN, qr=None):
                for q in qr if qr is not None else range(CPS):
                    c = CPS * s + q
                    m = c % HALF
                    lo = 0 if c < HALF else 32
                    nc.tensor.matmul(
                        psD[lo : lo + 32, :], selo[:, m * 32 : (m + 1) * 32],
                        eN[:, q * CH : (q + 1) * CH],
                        start=(m == 0), stop=(m == HALF - 1), skip_group_check=True,
                    )

            def nloop(s, eNL, qr=None):
                for q in qr if qr is not None else range(CPS):
                    c = CPS * s + q
                    m = c % HALF
                    lo = 0 if c < HALF else 32
                    nc.tensor.matmul(
                        psN[lo : lo + 32, :], seld[:, m * 32 : (m + 1) * 32],
                        eNL[:, q * CH : (q + 1) * CH],
                        start=(m == 0), stop=(m == HALF - 1), skip_group_check=True,
                    )

            def poly_chain(eng, rs):
                eng.tensor_scalar_mul(r[rs], tl[rs], float(POLY[0]))
                for cf in POLY[1:-1]:
                    eng.scalar_tensor_tensor(
                        r[rs], r[rs], float(cf), tl[rs], op0=ALU.add, op1=ALU.mult
                    )

            def prep_ops(eng, rs):
                """Off-critical pieces: rP = (r + P_last) * (1/den), and the
                valid-count column. Tail after the last num-matmul is then just
                qt = rP * psN -> reduce."""
                eng.scalar_tensor_tensor(
                    w[rs], r[rs], float(POLY[-1]), rd[rs], op0=ALU.add, op1=ALU.mult
                )
                nc.vector.reduce_sum(colv[rs], vt[rs], axis=AX.X)

            def tail_ops(eng, rs):
                eng.tensor_tensor(qt[rs], w[rs], psN[rs], op=ALU.mult)
                nc.vector.reduce_sum(colp[rs], qt[rs], axis=AX.X)

            for _rep in range(repeats):
                last = _rep == repeats - 1
                rsA, rsB = slice(0, 32), slice(32, 64)

                # ---- DMA stream (Pool SWDGE queue, this order) ----
                s16 = [
                    spool.tile([128, FREE], F16, name=f"s16_{i}", tag=f"s16_{i % 4}")
                    for i in range(SUPERS)
                ]
                l16 = [
                    lpool.tile([128, FREE], F16, name=f"l16_{i}", tag=f"l16_{i}")
                    for i in range(SUPERS)
                ]
                order = [("s", 0), ("l", 0), ("selo", 0), ("s", 1), ("l", 1),
                         ("s", 2), ("l", 2), ("s", 3), ("l", 3),
                         ("s", 4), ("l", 4), ("s", 5), ("l", 5), ("s", 6),
                         ("l", 6), ("l", 7), ("s7a", 0), ("s7b", 0)]
                for kind, i in order:
                    if kind == "s":
                        nc.gpsimd.dma_start(s16[i][:], scores_d[i])
                    elif kind == "l":
                        nc.gpsimd.dma_start(l16[i][:], labels_d[i])
                    elif kind == "selo":
                        if _rep == 0:
                            nc.gpsimd.dma_start(selo[:], selo_d[:])
                            nc.vector.tensor_scalar_mul(seld[:], selo[:], cvec[:, 0:1])
                    elif kind == "s7a":
                        nc.gpsimd.dma_start(s16[7][:, 0:HF], scores_d[7, :, 0:HF])
                    else:
                        nc.gpsimd.dma_start(s16[7][:, HF:FREE], scores_d[7, :, HF:FREE])

                # ---- main supers, half-tile compute granularity ----
                eN = {}
                eNL = {}
                for s in range(SUPERS):
                    eN[s] = epool.tile([128, FREE], F16, name=f"eN_{s}", tag=f"eN_{s % 3}")
                    eNL[s] = e2pool.tile([128, FREE], F16, name=f"eNL_{s}", tag=f"eNL_{s % 3}")
                    if s == 7 and last:
                        nc.vector.tensor_scalar_max(kc[rsB], psK[rsB], 1.0)
                        nc.vector.tensor_scalar(vt[rsB], psK[rsB], 0.5, None, op0=ALU.is_ge)
                    for h in range(2):
                        hsl = slice(h * HF, (h + 1) * HF)
                        nc.scalar.activation(
                            eN[s][:, hsl], s16[s][:, hsl], AF.Exp, bias=cvec[:, 1:2]
                        )
                        nc.vector.tensor_tensor(
                            eNL[s][:, hsl], eN[s][:, hsl], l16[s][:, hsl], op=ALU.mult
                        )
                        dloop(s, eN[s], qr=range(h * 4, h * 4 + 4))
                        nloop(s, eNL[s], qr=range(h * 4, h * 4 + 4))
                        if s == 7 and h == 0 and last:
                            # psK-B closed at kloop(7); Ln fits between exp7a/b
                            nc.scalar.activation(
                                tl[rsB], kc[rsB], AF.Ln, bias=cvec[32:64, 1:2]
                            )
                            poly_chain(nc.vector, rsB)
                        if h == 0 and s < 7:
                            # K-matmuls sit between the half-tiles: labels land
                            # just after their scores (K7 rides with super 6)
                            kloop(s, l16[s])
                            if s == 6:
                                kloop(7, l16[7])

                    if s == 3 and last:
                        # group A closed at the end of super 3
                        nc.vector.tensor_scalar_max(kc[rsA], psK[rsA], 1.0)
                        nc.scalar.activation(tl[rsA], kc[rsA], AF.Ln, bias=cvec[0:32, 1:2])
                        nc.vector.tensor_scalar(vt[rsA], psK[rsA], 0.5, None, op0=ALU.is_ge)
                        nc.vector.reciprocal(rd[rsA], psD[rsA])
                        poly_chain(nc.vector, rsA)
                    if s == 5 and last:
                        prep_ops(nc.vector, rsA)
                        tail_ops(nc.vector, rsA)  # group-A epilogue (PSUM: DVE only)

                if last:
                    nc.vector.reciprocal(rd[rsB], psD[rsB])
                    prep_ops(nc.vector, rsB)
                    tail_ops(nc.vector, rsB)  # tail-critical group-B epilogue

            outc = ph.tile([64, 2], F32, tag="p2outc")
            nc.vector.tensor_copy(outc[:, 0:1], colv[:])
            nc.vector.tensor_copy(outc[:, 1:2], colp[:])
            nc.sync.dma_start(out_d[:], outc[:])

    _split_drain_waits(nc)
    return nc


_NC_CACHE = None


def get_nc():
    global _NC_CACHE
    if _NC_CACHE is None:
        _NC_CACHE = build_nc()
    return _NC_CACHE


def make_in_maps(scores, labels):
    # per-core shard, then transpose so candidates sit on partitions:
    # [NCORES, SUPERS, 2048 graphs, 128 cand] -> [NCORES, SUPERS, 128, 2048]
    scores_sh = np.ascontiguousarray(
        np.asarray(scores, dtype=np.float32)
        .reshape(NCORES, SUPERS, FREE, G)
        .transpose(0, 1, 3, 2)
    )
    labels_sh = np.ascontiguousarray(
        np.asarray(labels, dtype=np.int32)
        .reshape(NCORES, SUPERS, FREE, G)
        .transpose(0, 1, 3, 2)
    )
    consts = _make_consts()
    selo = _make_selo()
    return [
        {"scores": scores_sh[c], "labels": labels_sh[c], "consts": consts, "selo": selo}
        for c in range(NCORES)
    ]


_RUNNER_CACHE = None


def _get_runner():
    """Compile the NEFF + jitted shard_map executor once per process."""
    global _RUNNER_CACHE
    if _RUNNER_CACHE is not None:
        return _RUNNER_CACHE

    import jax
    from jax.sharding import Mesh, PartitionSpec, NamedSharding
    from jax.experimental.shard_map import shard_map
    from concourse import bass2jax

    nc = get_nc()
    bass2jax.install_neuronx_cc_hook()
    partition_name = nc.partition_id_tensor.name if nc.partition_id_tensor else None
    in_names, out_names, out_avals, zero_outs = [], [], [], []
    for alloc in nc.m.functions[0].allocations:
        if not isinstance(alloc, mybir.MemoryLocationSet):
            continue
        name = alloc.memorylocations[0].name
        if alloc.kind == "ExternalInput":
            if name != partition_name:
                in_names.append(name)
        elif alloc.kind == "ExternalOutput":
            shape = tuple(alloc.tensor_shape)
            dtype = mybir.dt.np(alloc.dtype)
            out_names.append(name)
            out_avals.append(jax.core.ShapedArray(shape, dtype))
            zero_outs.append(np.zeros(shape, dtype))
    n_params = len(in_names)
    n_outs = len(out_avals)
    all_in_names = list(in_names) + list(out_names)
    if partition_name is not None:
        all_in_names.append(partition_name)

    def _body(*args):
        operands = list(args)
        if partition_name is not None:
            operands.append(bass2jax.partition_id_tensor())
        return tuple(
            bass2jax._bass_exec_p.bind(
                *operands,
                out_avals=tuple(out_avals),
                in_names=tuple(all_in_names),
                out_names=tuple(out_names),
                lowering_input_output_aliases=(),
                sim_require_finite=True,
                sim_require_nnan=True,
                nc=nc,
            )
        )

    devices = jax.devices()[:NCORES]
    mesh = Mesh(np.asarray(devices), ("core",))
    sharded = jax.jit(
        shard_map(
            _body,
            mesh=mesh,
            in_specs=(PartitionSpec("core"),) * (n_params + n_outs),
            out_specs=(PartitionSpec("core"),) * n_outs,
            check_rep=False,
        ),
        keep_unused=True,
    )
    sharding = NamedSharding(mesh, PartitionSpec("core"))

    def run(in_maps):
        concat_in = [
            np.concatenate(
                [np.asarray(in_maps[c][nm]) for c in range(NCORES)], axis=0
            )
            for nm in in_names
        ]
        concat_zeros = [
            np.zeros((NCORES * z.shape[0], *z.shape[1:]), z.dtype) for z in zero_outs
        ]
        dev_in = [jax.device_put(a, sharding) for a in concat_in]
        dev_zeros = [jax.device_put(a, sharding) for a in concat_zeros]
        outs = sharded(*dev_in, *dev_zeros)
        outs = [np.asarray(o) for o in outs]
        return {
            nm: outs[i].reshape(NCORES, *out_avals[i].shape) for i, nm in enumerate(out_names)
        }

    _RUNNER_CACHE = run
    return run


def reduce_out(out_concat):
    """[NCORES*64, 2] device output -> full loss sum: col0 = per-row valid
    counts, col1 = per-row sum of valid*ndcg; loss = sum(valid) - sum(ndcg)."""
    o = np.asarray(out_concat).reshape(NCORES, 64, 2)
    return float(o[..., 0].sum() - o[..., 1].sum())


def kernel(scores, labels, batch):
    run = _get_runner()
    in_maps = make_in_maps(scores, labels)
    outs = run(in_maps)
    total = reduce_out(outs["out"])
    return np.float32(total / B)


# revision 44
# speedup vs baseline: 1.2598x; 1.0079x over previous
"""ApproxNDCGLoss Trainium2 kernel v2 (8 NeuronCores, data-parallel over graphs).

Math (per graph of G=128 candidates, labels binary):
  probs    = softmax(scores)            (no max-subtract: scores ~ N(0,1), fp32-safe)
  edcg     = sum_j probs_j * l_j * disc_j,   disc_j = 1/log2(j+2)
  idcg     = C(k), k = sum_j l_j, C = cumsum(disc)   (descending sort of binary
             labels == k ones first, so no sort needed)
  loss_g   = [k>0] * (1 - edcg/idcg);  loss = sum_g loss_g / B

Layout: the host hands each core its shard pre-TRANSPOSED so candidates sit on
the partition axis: [SUPERS=8, 128 cand, 2048 graphs] (full-width f32/i32 —
the device still streams all 16 MiB/core from HBM; only the layout changed).
This removes all PE transposes (the v1 kernel burned ~43us of PE sequencer on
512 data-as-weights Ldweights).

Per super-tile [128c x 2048g]:
  - gpsimd DMA loads scores f32->fp16 and labels i32->fp16 (cast in DMA)
  - ACT: eN = exp(s16)                   (plain softmax numerator)
  - DVE: eNL = eN * l16                  (fp16 2x mode)
  - PE, selector-weight accumulation: chunk q of 256 graphs, c = 8s+q,
    SEL[c%32] is [128,32] with column (c%32) = ones (or disc), so
       matmul(psX[rowbase:rowbase+32], SEL, rhs_chunk, start/stop)
    accumulates row c of a compact [64 rows x 256 graphs] PSUM tile:
       psD row c = denom_g = sum_c eN          (ones selector)
       psN row c = num_g   = sum_c eNL * disc  (disc-scaled selector)
       psK row c = k_g     = sum_c l           (ones selector)
    Weights never carry data -> only ~2 small Ldweights per chunk and the
    per-graph scalars land compact for phase 2.
  - phase 2 (two row-groups, ops interleaved into idle engine windows):
    1/C(k) via degree-5 polynomial in ln k (max rel err 2.8e-3 vs 2e-2 tol),
    loss sum = sum(valid) - sum(num*poly(ln k)/denom) per row; the per-row
    [64,2] (valid-count, ndcg-sum) columns are DMA'd out and summed on host.
  - a short warm-up matmul burst ramps the PE clock (0.65->2.4 GHz pstate)
    before the real stream arrives.
Host: shard + transpose inputs, combine 8x[64,2] partials, / B.
`batch` is repeat(arange(B), G) by construction and is never read.
"""

import sys
from contextlib import ExitStack

import numpy as np

TRN_REPO = "/opt/trn_rl_repo"
if TRN_REPO not in sys.path:
    sys.path.insert(0, TRN_REPO)

import concourse.bass as bass
import concourse.mybir as mybir
import concourse.tile as tile

B = 131072
G = 128
NCORES = 8
BPC = B // NCORES            # graphs per core (16384)
SUPERS = 8                   # super-tiles per core
FREE = BPC // SUPERS         # graphs per super-tile (2048)
CH = 256                     # graphs per PE chunk
CPS = FREE // CH             # chunks per super (8)
NCHUNK = BPC // CH           # chunks per core (64) == compact rows
HALF = NCHUNK // 2           # accumulation-group boundary (32)
NWARM = 28                   # PE warm-up matmuls (pstate ramp)

F32 = mybir.dt.float32
F16 = mybir.dt.float16
I32 = mybir.dt.int32


def _fit_poly():
    """Degree-5 poly p(t) ~= 1/C(e^t), t = ln k (max rel err 2.8e-3 at
    k = 1..128; loss tolerance is 2e-2)."""
    disc = 1.0 / np.log2(np.arange(1, G + 1, dtype=np.float64) + 1.0)
    C = np.cumsum(disc)
    k = np.arange(1, G + 1, dtype=np.float64)
    t = np.log(k)
    g = 1.0 / C
    w = 1.0 / g
    deg = 5
    for _ in range(60):
        cf = np.polyfit(t, g, deg, w=w)
        rel = (np.polyval(cf, t) - g) / g
        w = w * (1 + 3 * np.abs(rel) / np.abs(rel).max())
    return [float(c) for c in cf]


POLY = _fit_poly()


def _make_consts():
    # disc_j for 0-based candidate j is 1/log2(j+2)
    disc = 1.0 / np.log2(np.arange(1, G + 1, dtype=np.float64) + 1.0)
    consts = np.zeros((128, 4), dtype=np.float32)
    consts[:, 0] = disc
    # col 2: Ln bias. ln(k + 1e-30) == ln(k) exactly in f32 for k >= 1; for
    # k == 0 it gives ln(1e-30) = -69, whose finite poly extrapolation is
    # annihilated later by qt = w * psN (psN == 0 when no labels).
    consts[:, 2] = 1e-30
    return consts


def _make_selo():
    """[128, 32*32] fp16: tile m*32.. holds the ones-selector for row (c%32):
    selo[p, 32*m + j] = 1.0 iff j == m."""
    selo = np.zeros((128, HALF * HALF), dtype=np.float16)
    for m in range(HALF):
        selo[:, HALF * m + m] = 1.0
    return selo


def _split_drain_waits(nc, max_waits=1):
    """Workaround: this neuronxcc build rejects instructions carrying more
    than ~1 sem wait ("Too many sync wait commands"). Hoist excess waits
    onto standalone InstEventSemaphore instructions issued immediately
    before, on the same engine queue (in-order, so semantics unchanged)."""
    ctr = 0
    for f in nc.m.functions:
        for blk in f.blocks:
            new_list = []
            for inst in blk.instructions:
                si = inst.sync_info
                if (
                    si is not None
                    and si.on_wait
                    and len(si.on_wait) > max_waits
                    and not isinstance(inst, mybir.InstEventSemaphore)
                ):
                    keep = si.on_wait[-max_waits:]
                    for wt in si.on_wait[:-max_waits]:
                        ctr += 1
                        ev = mybir.InstEventSemaphore(
                            name=f"hoistwait-{ctr}",
                            ins=[],
                            outs=[],
                            sync_info=mybir.SyncInfo(on_wait=[wt], on_update=[]),
                        )
                        ev.engine = inst.engine
                        new_list.append(ev)
                    si.on_wait = keep
                new_list.append(inst)
            blk.instructions = new_list


def build_nc(repeats=1):
    """repeats>1 unrolls the main pipeline R times over the same data
    (identical results) — used only for device-time measurement."""
    AF = mybir.ActivationFunctionType
    ALU = mybir.AluOpType
    AX = mybir.AxisListType

    nc = bass.Bass("TRN2", target_bir_lowering=False, debug=False, num_devices=NCORES)
    scores_d = nc.dram_tensor("scores", [SUPERS, 128, FREE], F32, kind="ExternalInput").ap()
    labels_d = nc.dram_tensor("labels", [SUPERS, 128, FREE], I32, kind="ExternalInput").ap()
    consts_d = nc.dram_tensor("consts", [128, 4], F32, kind="ExternalInput").ap()
    selo_d = nc.dram_tensor("selo", [128, HALF * HALF], F16, kind="ExternalInput").ap()
    out_d = nc.dram_tensor("out", [64, 2], F32, kind="ExternalOutput").ap()

    with tile.TileContext(nc) as tc:
        with ExitStack() as ctx:
            cpool = ctx.enter_context(tc.tile_pool(name="consts", bufs=1))
            cvec = cpool.tile([128, 4], F32)
            nc.sync.dma_start(cvec[:], consts_d[:])
            selo = cpool.tile([128, HALF * HALF], F16)
            seld = cpool.tile([128, HALF * HALF], F16)
            # PE pstate warm-up scratch
            wsrc = cpool.tile([128, HALF], F16)
            nc.vector.memset(wsrc[:], 0.0)
            rsrc = cpool.tile([128, CH], F16)
            nc.vector.memset(rsrc[:], 0.0)

            # compact per-graph scalars: group A rows [0:32] (chunks 0-31),
            # group B rows [32:64] (chunks 32-63) — PE output partition base
            # must be 0/32/64/96, so the two groups sit at bases 0 and 32
            pdp = ctx.enter_context(tc.tile_pool(name="cd", bufs=1, space="PSUM"))
            psD = pdp.tile([64, CH], F32)
            pnp = ctx.enter_context(tc.tile_pool(name="cn", bufs=1, space="PSUM"))
            psN = pnp.tile([64, CH], F32)
            pkp = ctx.enter_context(tc.tile_pool(name="ck", bufs=1, space="PSUM"))
            psK = pkp.tile([64, CH], F32)
            pwp = ctx.enter_context(tc.tile_pool(name="scr", bufs=1, space="PSUM"))
            pscr = pwp.tile([32, CH], F32)

            spool = ctx.enter_context(tc.tile_pool(name="s16", bufs=1))
            lpool = ctx.enter_context(tc.tile_pool(name="l16", bufs=1))
            epool = ctx.enter_context(tc.tile_pool(name="eN", bufs=1))
            e2pool = ctx.enter_context(tc.tile_pool(name="eNL", bufs=1))
            ph = ctx.enter_context(tc.tile_pool(name="ph", bufs=1))

            # phase-2 tiles, shared by the two row-group passes
            tl = ph.tile([64, CH], F32, tag="p2tl")
            r = ph.tile([64, CH], F32, tag="p2r")
            w = ph.tile([64, CH], F32, tag="p2w")
            rd = ph.tile([64, CH], F32, tag="p2rd")
            qt = ph.tile([64, CH], F32, tag="p2qt")
            vt = ph.tile([64, CH], F32, tag="p2vt")
            colp = ph.tile([64, 1], F32, tag="p2colp")
            colv = ph.tile([64, 1], F32, tag="p2colv")

            # PE warm-up: ramp the tensor engine to full clock before the
            # real matmul stream arrives (cold-start runs at 0.65/1.2 GHz)
            for _wi in range(NWARM):
                nc.tensor.matmul(pscr[:], wsrc[:], rsrc[:], start=True, stop=True)

            HF = FREE // 2

            def kloop(s, l16):
                for q in range(CPS):
                    c = CPS * s + q
                    m = c % HALF
                    lo = 0 if c < HALF else 32
                    nc.tensor.matmul(
                        psK[lo : lo + 32, :], selo[:, m * 32 : (m + 1) * 32],
                        l16[:, q * CH : (q + 1) * CH],
                        start=(m == 0), stop=(m == HALF - 1), skip_group_check=True,
                    )

            def dloop(s, eN, qr=None):
                for q in qr if qr is not None else range(CPS):
                    c = CPS * s + q
                    m = c % HALF
                    lo = 0 if c < HALF else 32
                    nc.tensor.matmul(
                        psD[lo : lo + 32, :], selo[:, m * 32 : (m + 1) * 32],
                        eN[:, q * CH : (q + 1) * CH],
                        start=(m == 0), stop=(m == HALF - 1), skip_group_check=True,
                    )

            def nloop(s, eNL, qr=None):
                for q in qr if qr is not None else range(CPS):
                    c = CPS * s + q
                    m = c % HALF
                    lo = 0 if c < HALF else 32
                    nc.tensor.matmul(
                        psN[lo : lo + 32, :], seld[:, m * 32 : (m + 1) * 32],
                        eNL[:, q * CH : (q + 1) * CH],
                        start=(m == 0), stop=(m == HALF - 1), skip_group_check=True,
                    )

            def poly_chain(eng, rs):
                eng.tensor_scalar_mul(r[rs], tl[rs], float(POLY[0]))
                for cf in POLY[1:-1]:
                    eng.scalar_tensor_tensor(
                        r[rs], r[rs], float(cf), tl[rs], op0=ALU.add, op1=ALU.mult
                    )

            def prep_ops(eng, rs):
                """Off-critical pieces: rP = (r + P_last) * (1/den), and the
                valid-count column. Tail after the last num-matmul is then just
                qt = rP * psN -> reduce."""
                eng.scalar_tensor_tensor(
                    w[rs], r[rs], float(POLY[-1]), rd[rs], op0=ALU.add, op1=ALU.mult
                )
                nc.vector.reduce_sum(colv[rs], vt[rs], axis=AX.X)

            def tail_ops(eng, rs):
                eng.tensor_tensor(qt[rs], w[rs], psN[rs], op=ALU.mult)
                nc.vector.reduce_sum(colp[rs], qt[rs], axis=AX.X)

            for _rep in range(repeats):
                last = _rep == repeats - 1
                rsA, rsB = slice(0, 32), slice(32, 64)

                # ---- DMA stream (Pool SWDGE queue, this order) ----
                s16 = [
                    spool.tile([128, FREE], F16, name=f"s16_{i}", tag=f"s16_{i % 4}")
                    for i in range(SUPERS)
                ]
                l16 = [
                    lpool.tile([128, FREE], F16, name=f"l16_{i}", tag=f"l16_{i}")
                    for i in range(SUPERS)
                ]
                order = [("s", 0), ("l", 0), ("selo", 0), ("s", 1), ("l", 1),
                         ("s", 2), ("l", 2), ("s", 3), ("l", 3),
                         ("s", 4), ("l", 4), ("s", 5), ("l", 5), ("s", 6),
                         ("l", 6), ("l", 7), ("s7a", 0), ("s7b", 0)]
                for kind, i in order:
                    if kind == "s":
                        nc.gpsimd.dma_start(s16[i][:], scores_d[i])
                    elif kind == "l":
                        nc.gpsimd.dma_start(l16[i][:], labels_d[i])
                    elif kind == "selo":
                        if _rep == 0:
                            nc.gpsimd.dma_start(selo[:], selo_d[:])
                            nc.vector.tensor_scalar_mul(seld[:], selo[:], cvec[:, 0:1])
                    elif kind == "s7a":
                        nc.gpsimd.dma_start(s16[7][:, 0:HF], scores_d[7, :, 0:HF])
                    else:
                        nc.gpsimd.dma_start(s16[7][:, HF:FREE], scores_d[7, :, HF:FREE])

                # ---- main supers, half-tile compute granularity ----
                eN = {}
                eNL = {}
                for s in range(SUPERS):
                    eN[s] = epool.tile([128, FREE], F16, name=f"eN_{s}", tag=f"eN_{s % 3}")
                    eNL[s] = e2pool.tile([128, FREE], F16, name=f"eNL_{s}", tag=f"eNL_{s % 3}")
                    if s == 7 and last:
                        nc.vector.tensor_scalar(vt[rsB], psK[rsB], 0.5, None, op0=ALU.is_ge)
                    for h in range(2):
                        hsl = slice(h * HF, (h + 1) * HF)
                        nc.scalar.activation(
                            eN[s][:, hsl], s16[s][:, hsl], AF.Exp, bias=cvec[:, 1:2]
                        )
                        nc.vector.tensor_tensor(
                            eNL[s][:, hsl], eN[s][:, hsl], l16[s][:, hsl], op=ALU.mult
                        )
                        dloop(s, eN[s], qr=range(h * 4, h * 4 + 4))
                        nloop(s, eNL[s], qr=range(h * 4, h * 4 + 4))
                        if s == 7 and h == 0 and last:
                            # psK-B closed at kloop(7); Ln fits between exp7a/b
                            nc.scalar.activation(
                                tl[rsB], psK[rsB], AF.Ln, bias=cvec[32:64, 2:3]
                            )
                            poly_chain(nc.vector, rsB)
                        if h == 0 and s < 7:
                            # K-matmuls sit between the half-tiles: labels land
                            # just after their scores (K7 rides with super 6)
                            kloop(s, l16[s])
                            if s == 6:
                                kloop(7, l16[7])

                    if s == 3 and last:
                        # group A closed at the end of super 3
                        nc.scalar.activation(tl[rsA], psK[rsA], AF.Ln, bias=cvec[0:32, 2:3])
                        nc.vector.tensor_scalar(vt[rsA], psK[rsA], 0.5, None, op0=ALU.is_ge)
                        nc.vector.reciprocal(rd[rsA], psD[rsA])
                        poly_chain(nc.vector, rsA)
                    if s == 5 and last:
                        prep_ops(nc.vector, rsA)
                        tail_ops(nc.vector, rsA)  # group-A epilogue (PSUM: DVE only)

                if last:
                    nc.vector.reciprocal(rd[rsB], psD[rsB])
                    prep_ops(nc.vector, rsB)
                    tail_ops(nc.vector, rsB)  # tail-critical group-B epilogue

            outc = ph.tile([64, 2], F32, tag="p2outc")
            nc.vector.tensor_copy(outc[:, 0:1], colv[:])
            nc.vector.tensor_copy(outc[:, 1:2], colp[:])
            nc.sync.dma_start(out_d[:], outc[:])

    _split_drain_waits(nc)
    return nc


_NC_CACHE = None


def get_nc():
    global _NC_CACHE
    if _NC_CACHE is None:
        _NC_CACHE = build_nc()
    return _NC_CACHE


def make_in_maps(scores, labels):
    # per-core shard, then transpose so candidates sit on partitions:
    # [NCORES, SUPERS, 2048 graphs, 128 cand] -> [NCORES, SUPERS, 128, 2048]
    scores_sh = np.ascontiguousarray(
        np.asarray(scores, dtype=np.float32)
        .reshape(NCORES, SUPERS, FREE, G)
        .transpose(0, 1, 3, 2)
    )
    labels_sh = np.ascontiguousarray(
        np.asarray(labels, dtype=np.int32)
        .reshape(NCORES, SUPERS, FREE, G)
        .transpose(0, 1, 3, 2)
    )
    consts = _make_consts()
    selo = _make_selo()
    return [
        {"scores": scores_sh[c], "labels": labels_sh[c], "consts": consts, "selo": selo}
        for c in range(NCORES)
    ]


_RUNNER_CACHE = None


def _get_runner():
    """Compile the NEFF + jitted shard_map executor once per process."""
    global _RUNNER_CACHE
    if _RUNNER_CACHE is not None:
        return _RUNNER_CACHE

    import jax
    from jax.sharding import Mesh, PartitionSpec, NamedSharding
    from jax.experimental.shard_map import shard_map
    from concourse import bass2jax

    nc = get_nc()
    bass2jax.install_neuronx_cc_hook()
    partition_name = nc.partition_id_tensor.name if nc.partition_id_tensor else None
    in_names, out_names, out_avals, zero_outs = [], [], [], []
    for alloc in nc.m.functions[0].allocations:
        if not isinstance(alloc, mybir.MemoryLocationSet):
            continue
        name = alloc.memorylocations[0].name
        if alloc.kind == "ExternalInput":
            if name != partition_name:
                in_names.append(name)
        elif alloc.kind == "ExternalOutput":
            shape = tuple(alloc.tensor_shape)
            dtype = mybir.dt.np(alloc.dtype)
            out_names.append(name)
            out_avals.append(jax.core.ShapedArray(shape, dtype))
            zero_outs.append(np.zeros(shape, dtype))
    n_params = len(in_names)
    n_outs = len(out_avals)
    all_in_names = list(in_names) + list(out_names)
    if partition_name is not None:
        all_in_names.append(partition_name)

    def _body(*args):
        operands = list(args)
        if partition_name is not None:
            operands.append(bass2jax.partition_id_tensor())
        return tuple(
            bass2jax._bass_exec_p.bind(
                *operands,
                out_avals=tuple(out_avals),
                in_names=tuple(all_in_names),
                out_names=tuple(out_names),
                lowering_input_output_aliases=(),
                sim_require_finite=True,
                sim_require_nnan=True,
                nc=nc,
            )
        )

    devices = jax.devices()[:NCORES]
    mesh = Mesh(np.asarray(devices), ("core",))
    sharded = jax.jit(
        shard_map(
            _body,
            mesh=mesh,
            in_specs=(PartitionSpec("core"),) * (n_params + n_outs),
            out_specs=(PartitionSpec("core"),) * n_outs,
            check_rep=False,
        ),
        keep_unused=True,
    )
    sharding = NamedSharding(mesh, PartitionSpec("core"))

    def run(in_maps):
        concat_in = [
            np.concatenate(
                [np.asarray(in_maps[c][nm]) for c in range(NCORES)], axis=0
            )
            for nm in in_names
        ]
        concat_zeros = [
            np.zeros((NCORES * z.shape[0], *z.shape[1:]), z.dtype) for z in zero_outs
        ]
        dev_in = [jax.device_put(a, sharding) for a in concat_in]
        dev_zeros = [jax.device_put(a, sharding) for a in concat_zeros]
        outs = sharded(*dev_in, *dev_zeros)
        outs = [np.asarray(o) for o in outs]
        return {
            nm: outs[i].reshape(NCORES, *out_avals[i].shape) for i, nm in enumerate(out_names)
        }

    _RUNNER_CACHE = run
    return run


def reduce_out(out_concat):
    """[NCORES*64, 2] device output -> full loss sum: col0 = per-row valid
    counts, col1 = per-row sum of valid*ndcg; loss = sum(valid) - sum(ndcg)."""
    o = np.asarray(out_concat).reshape(NCORES, 64, 2)
    return float(o[..., 0].sum() - o[..., 1].sum())


def kernel(scores, labels, batch):
    run = _get_runner()
    in_maps = make_in_maps(scores, labels)
    outs = run(in_maps)
    total = reduce_out(outs["out"])
    return np.float32(total / B)
